# revision 29
# baseline (speedup 1.0000x reference)
"""Trainium2 Bass kernel for nn_AugmentedAffinityContrastive.

loss = sum_i mean_{b,h,w}[ (1 - <embeds, roll(tf_embeds, off_i)>_E) * m_i * (0.5 - a_i) ]

Let c_i[h,w] = m_i[h,w] * (0.5 - a_i[h,w])  (batch-free, computed on host from raw
-- ~1MB of work vs 256MB for the embedding term).  Then

loss = (B * sum(c) - S) / (B*H*W),
S    = sum_{b,e,h,w} embeds[b,e,h,w] * T[b,e,h,w],
T    = sum_i c_i[h,w] * tf[b,e,(h-dy_i)%H,(w-dx_i)%W]

S is linear over the B*E = 128 (b,e) planes -> 16 planes per core across 8 cores.

Device pipeline per plane (all tensors fp16, accumulation fp32):
  - DVE:  7 of the 9 products c_i * shift_i(tf) as 3 paired + 1 single
          tensor_tensor (2x_1p mode; a pair shares one instruction via an
          overlapping strided tf view -- same dx, different dy)
  - Pool: the 2 remaining dx=0 products (tensor_tensor)
  - PE:   T = sum of the 9 product tiles via identity-matmul PSUM accumulation
  - Act:  T16 = fp16 copy of T out of PSUM
  - DVE:  junk = T16 * e (fast fp16 multiply)
  - Act:  accumulating copy of junk -> per-partition parts column
The tf halo is padded left in w so no product needs a circular-wrap fixup op.
Host combines: loss = (B*csum - sum(parts))/BHW.
"""

import numpy as np

OFFS = [[0, -1], [-1, 0], [-1, -1], [0, -2], [-2, 0], [-2, -2], [0, -3], [-3, 0], [-3, -3]]
SIGMA = 1.2
B, E, H, W = 4, 32, 512, 512
NCORES = 8
PLANES = B * E          # 128
PPC = PLANES // NCORES  # 16 planes per core
P = 128                 # partitions
HB = H // P             # 4 row chunks per partition; h = p*HB + hb
HALO = 7                # halo rows per partition: 4p-3 .. 4p+3
WPAD = 4                # left pad in w (3 used + 1 alignment spare)
WP = W + WPAD           # padded row length

# (dy, dx) = (-off[0], -off[1]); rolled[h,w] = tf[h-dy, w-dx]
DYDX = [(-o[0], -o[1]) for o in OFFS]
# c-map storage order (position -> offset index).  Positions 2k,2k+1 form a
# DVE pair computed by ONE tensor_tensor with an overlapping tf view (same
# dx, different dy); position 6 is the DVE single; 7,8 go to Pool.
CPOS = [2, 0, 5, 3, 8, 6, 1, 4, 7]
VPP = 5                  # vsem incs per plane (3 pairs + 1 single + 1 mult)

_CACHE = {}


def _blur_axis_np(x, k, r, axis):
    pad = [(0, 0)] * x.ndim
    pad[axis] = (r, r)
    xp = np.pad(x, pad, mode='edge')
    n = x.shape[axis]

    def sl(i):
        idx = [slice(None)] * x.ndim
        idx[axis] = slice(i, i + n)
        return xp[tuple(idx)]

    out = (k[0] * sl(0)).astype(np.float32)
    for i in range(1, 2 * r + 1):
        out = out + k[i] * sl(i)
    return out


def _host_cmaps(raw, mask):
    """Replicate reference's gaussian_blur + naive_affinities, return
    c[9,H,W] f32 with c_i = mask_i * (0.5 - a_i)."""
    r = int(4.0 * SIGMA + 0.5)
    t = np.arange(-r, r + 1)
    k = np.exp(-0.5 * (t / SIGMA) ** 2)
    k = (k / k.sum()).astype(np.float32)

    x = raw[0].astype(np.float32)          # [1,H,W]
    x = _blur_axis_np(_blur_axis_np(x, k, r, 1), k, r, 2)

    cs = []
    for i, off in enumerate(OFFS):
        rolled = np.roll(x, (-off[0], -off[1]), axis=(-2, -1))
        d = np.sqrt(((x - rolled) ** 2).sum(0))
        a = d / d.max()
        a = np.clip(a, 0.0, 1.0)
        a = a - a.min()
        a = a / a.max()
        cs.append(mask[0, i] * (np.float32(0.5) - a))
    return np.ascontiguousarray(np.stack(cs).astype(np.float32))


def _build_bass():
    """Raw-Bass SPMD program (manual semaphores; every instruction carries at
    most one semaphore wait -- the walrus build caps wait counts).

    Semaphores:
      dsem: DMA completions (sync engine issues all DMAs, +16 each)
      vsem: DVE progress, VPP=5 per plane (3 pairs + single + mult)
      gsem: Pool progress, 2 per plane (2 products)
      psem: PE progress, 1 per plane (full T accumulated)
      asem: Act progress, 2 per plane (T16 copy + accumulating reduce)

    DVE runs software-pipelined: products(q+1) are issued before mult(q), so
    DVE never stalls on the PE tail.  vsem milestones (q >= 1):
      product instr k (1..4) of plane q ends at 5q - 1 + k
      mult(q) ends at 5q + 9 (q <= 14), 80 for q = 15
    """
    import concourse.bass as bass
    import concourse.mybir as mybir
    from concourse.ap import AP

    f32 = mybir.dt.float32
    f16 = mybir.dt.float16
    MUL = mybir.AluOpType.mult
    COPY = mybir.ActivationFunctionType.Copy

    nc = bass.Bass()
    e_in = nc.dram_tensor("e_sh", [PPC, H, W], f16, kind="ExternalInput")
    # host-prepared padded halo: tf_in[q, p, c, j] = tf[q, (4p+c-3)%H, (j-3)%W]
    tf_in = nc.dram_tensor("tf_sh", [PPC, P, HALO, WP], f16, kind="ExternalInput")
    # host-prepared, CPOS order: c_in[p, pos, hb, w] = c_{CPOS[pos]}[p*HB+hb, w]
    c_in = nc.dram_tensor("cmap", [P, 9, HB, W], f16, kind="ExternalInput")
    id_in = nc.dram_tensor("ident", [P, P], f16, kind="ExternalInput")
    parts_out = nc.dram_tensor("partials", [P, PPC], f32, kind="ExternalOutput")

    NS = 3   # input slots
    NT = 2   # product-tile slots
    NPS = 2  # psum T slots (PSUM only fits 2 x [P, 2048] f32)

    # DVE pair k covers positions 2k, 2k+1 (same dx, dy_a > dy_b)
    pairs = []
    for k in range(3):
        (dya, dxa) = DYDX[CPOS[2 * k]]
        (dyb, dxb) = DYDX[CPOS[2 * k + 1]]
        assert dxa == dxb and dya > dyb
        pairs.append((dya, dyb, dxa))
    sdy, sdx = DYDX[CPOS[6]]                      # the DVE single, (1, 0)
    pool_dydx = [DYDX[CPOS[7]], DYDX[CPOS[8]]]    # (2, 0), (3, 0)

    with (
        nc.sbuf_tensor([P, 9 * HB * W], f16) as ct_s,
        nc.sbuf_tensor([P, NS * HB * W], f16) as e_s,
        nc.sbuf_tensor([P, NS * HALO * WP], f16) as tf_s,
        nc.sbuf_tensor([P, NT * 7 * HB * W], f16) as tmpd_s,
        nc.sbuf_tensor([P, NT * 2 * HB * W], f16) as tmpp_s,
        nc.sbuf_tensor([P, NT * HB * W], f16) as t16_s,
        nc.sbuf_tensor([P, NT * HB * W], f16) as junk_s,
        nc.sbuf_tensor([P, HB * W], f16) as dump_s,
        nc.sbuf_tensor([P, P], f16) as id_s,
        nc.sbuf_tensor([P, PPC], f32) as parts_s,
        nc.psum_tensor([P, NPS * HB * W], f32) as t_ps,
        nc.semaphore() as dsem,
        nc.semaphore() as vsem,
        nc.semaphore() as gsem,
        nc.semaphore() as psem,
        nc.semaphore() as asem,
        nc.Block() as block,
    ):
        ct_v = ct_s[:].rearrange("p (i hb w) -> p i hb w", i=9, w=W)
        e_v = e_s[:].rearrange("p (s c w) -> p s c w", s=NS, w=W)
        tf_v = tf_s[:].rearrange("p (s c w) -> p s c w", s=NS, w=WP)
        tmpd_v = tmpd_s[:].rearrange("p (s r w) -> p s r w", s=NT, w=HB * W)
        tmpp_v = tmpp_s[:].rearrange("p (s r w) -> p s r w", s=NT, w=HB * W)
        t16_v = t16_s[:].rearrange("p (s w) -> p s w", s=NT)
        junk_v = junk_s[:].rearrange("p (s w) -> p s w", s=NT)
        t_v = t_ps[:].rearrange("p (s w) -> p s w", s=NPS)

        TFROW = NS * HALO * WP  # tf_s per-partition row length (elements)

        def tf_ap(sl, dy, dx):
            # [P, HB, W] view of tf slot sl shifted by (dy, dx):
            # value at (p, hb, w) = tf[4p + hb - dy, w - dx]  (circular)
            return tf_v[:, sl, 3 - dy:3 - dy + HB, 3 - dx:3 - dx + W]

        def tf_pair_ap(sl, k):
            # [P, 2, HB, W] overlapping view: pair dim steps dy_a -> dy_b
            dya, dyb, dx = pairs[k]
            off = sl * (HALO * WP) + (3 - dya) * WP + (3 - dx)
            return AP(tf_s[:].tensor, off,
                      [[TFROW, P], [(dya - dyb) * WP, 2], [WP, HB], [1, W]])

        # DMA issue order (1-based dsem counts, x16 each):
        #  1 tf0 | 2 ident | 3 ct pair0 | 4 ct pool | 5 ct pair1 | 6 tf1
        #  7 ct pair2 | 8 ct single | 9 e0 | 10 tf2 | 11 e1 | 12 e2
        #  then per plane q>=3: e(q) at 2q+7, tf(q) at 2q+8; parts last.
        N_DMA = 12 + 2 * (PPC - 3) + 1

        def dma_pos(q):  # dsem count at which plane q's inputs are resident
            return {0: 8, 1: 6, 2: 10}.get(q, 2 * q + 8)

        @block.sync
        def _(sync):
            sync.dma_start(tf_v[:, 0], tf_in[0]).then_inc(dsem, 16)
            sync.dma_start(id_s[:], id_in[:]).then_inc(dsem, 16)
            sync.dma_start(ct_v[:, 0:2], c_in[:, 0:2]).then_inc(dsem, 16)
            sync.dma_start(ct_v[:, 7:9], c_in[:, 7:9]).then_inc(dsem, 16)
            sync.dma_start(ct_v[:, 2:4], c_in[:, 2:4]).then_inc(dsem, 16)
            sync.dma_start(tf_v[:, 1], tf_in[1]).then_inc(dsem, 16)
            sync.dma_start(ct_v[:, 4:6], c_in[:, 4:6]).then_inc(dsem, 16)
            sync.dma_start(ct_v[:, 6:7], c_in[:, 6:7]).then_inc(dsem, 16)
            sync.dma_start(
                e_v[:, 0],
                e_in[0].rearrange("(p hb) w -> p hb w", hb=HB),
            ).then_inc(dsem, 16)
            sync.dma_start(tf_v[:, 2], tf_in[2]).then_inc(dsem, 16)
            for q in (1, 2):
                sync.dma_start(
                    e_v[:, q],
                    e_in[q].rearrange("(p hb) w -> p hb w", hb=HB),
                ).then_inc(dsem, 16)
            for q in range(NS, PPC):
                sl = q % NS
                # e slot q%NS last read by mult(q-NS), ending at vsem 5q-6;
                # that also implies products(q-NS) are done with the tf slot
                # and (via copy1(q-NS)) that PE(q-NS+1) finished the tiles.
                sync.wait_ge(vsem, 5 * q - 6)
                sync.dma_start(
                    e_v[:, sl],
                    e_in[q].rearrange("(p hb) w -> p hb w", hb=HB),
                ).then_inc(dsem, 16)
                # Pool side of the tf slot needs its own count
                sync.wait_ge(gsem, 2 * (q - NS + 1))
                sync.dma_start(tf_v[:, sl], tf_in[q]).then_inc(dsem, 16)
            sync.wait_ge(asem, 2 * PPC)
            sync.dma_start(parts_out[:], parts_s[:]).then_inc(dsem, 16)
            sync.wait_ge(dsem, 16 * N_DMA)  # all DMAs complete

        @block.vector
        def _(vector):
            # plane-0/1 per-instruction DMA gates (ct pieces land piecemeal)
            #   plane 0: pair0 after DMA 3, pair1 after 5, pair2 after 7,
            #            single after 8
            #   plane 1: pair0 after 6 (tf1; ct pair0/1 already earlier),
            #            pair2 after 7, single after 8
            EARLY = {0: {0: 3, 1: 5, 2: 7, 3: 8}, 1: {0: 6, 2: 7, 3: 8}}

            def products(q):
                sl = q % NS
                gates = EARLY.get(q, {0: dma_pos(q)})
                for k in range(3):
                    if k in gates:
                        vector.wait_ge(dsem, 16 * gates[k])
                    vector.tensor_tensor(
                        tmpd_v[:, q % NT, 2 * k:2 * k + 2].rearrange(
                            "p r (hb w) -> p r hb w", w=W),
                        ct_v[:, 2 * k:2 * k + 2], tf_pair_ap(sl, k), MUL,
                    ).then_inc(vsem, 1)
                if 3 in gates:
                    vector.wait_ge(dsem, 16 * gates[3])
                vector.tensor_tensor(
                    tmpd_v[:, q % NT, 6].rearrange("p (hb w) -> p hb w", w=W),
                    ct_v[:, 6], tf_ap(sl, sdy, sdx), MUL,
                ).then_inc(vsem, 1)

            def mult(q):
                # junk = T16 * e; Act's copy1 (asem tick 2q+1) produced T16
                vector.wait_ge(asem, 2 * q + 1)
                vector.tensor_tensor(
                    junk_v[:, q % NT], t16_v[:, q % NT],
                    e_v[:, q % NS].rearrange("p c w -> p (c w)"), MUL,
                ).then_inc(vsem, 1)

            products(0)
            for q in range(1, PPC):
                products(q)
                mult(q - 1)
            mult(PPC - 1)

        @block.gpsimd
        def _(gpsimd):
            for q in range(PPC):
                sl = q % NS
                for j, (dy, dx) in enumerate(pool_dydx):
                    if j == 0:
                        # plane q inputs resident (plane 0: ct pool maps at
                        # DMA 4; tf0 even earlier)
                        gpsimd.wait_ge(dsem, 16 * (4 if q == 0 else dma_pos(q)))
                    gpsimd.tensor_tensor(
                        tmpp_v[:, q % NT, j].rearrange(
                            "p (hb w) -> p hb w", w=W),
                        ct_v[:, 7 + j], tf_ap(sl, dy, dx), MUL,
                    ).then_inc(gsem, 1)

        @block.tensor
        def _(tensor):
            # tile -> DVE instruction rank (1..4) producing it
            RANK = {0: 1, 1: 1, 2: 2, 3: 2, 4: 3, 5: 3, 6: 4}
            # consumption order: by availability; pool tiles (7, 8) later
            ORDER = [0, 1, 2, 3, 7, 4, 5, 6, 8]
            for q in range(PPC):
                for g, t in enumerate(ORDER):
                    # group 0 is a DVE tile: its vsem wait implies mult(q-2)
                    # -> copy1(q-2) done, so psum slot q%NPS is free
                    if t <= 6:
                        tensor.wait_ge(
                            vsem,
                            RANK[t] if q == 0 else 5 * q - 1 + RANK[t])
                    else:
                        tensor.wait_ge(gsem, 2 * q + (t - 6))
                    rhs_tile = (tmpd_v[:, q % NT, t] if t <= 6
                                else tmpp_v[:, q % NT, t - 7])
                    for c in range(HB):
                        ins = tensor.matmul(
                            t_v[:, q % NPS, c * W:(c + 1) * W],
                            id_s[:], rhs_tile[:, c * W:(c + 1) * W],
                            start=(g == 0), stop=(g == 8),
                        )
                        if c == HB - 1 and g == 8:
                            ins.then_inc(psem, 1)

        @block.scalar
        def _(scalar):
            for q in range(PPC):
                # T16 = fp16 copy of the finished T (frees the psum slot)
                scalar.wait_ge(psem, q + 1)
                scalar.activation(
                    t16_v[:, q % NT], t_v[:, q % NPS], COPY,
                ).then_inc(asem, 1)
                # accumulating reduce of junk = T16*e into the parts column
                scalar.wait_ge(
                    vsem, 5 * q + 9 if q < PPC - 1 else VPP * PPC)
                scalar.activation(
                    dump_s[:], junk_v[:, q % NT], COPY,
                    accum_out=parts_s[:, q:q + 1],
                ).then_inc(asem, 1)
    return nc


def _prepare(embeds, tf_embeds, raw, mask):
    c = _host_cmaps(np.asarray(raw, np.float32), np.asarray(mask, np.float32))
    csum = c.sum(dtype=np.float64)
    # device layout in CPOS order: cl[p, pos, hb, w] = c_{CPOS[pos]}[p*HB+hb, w]
    cl = np.ascontiguousarray(
        c[CPOS].reshape(9, P, HB, W).transpose(1, 0, 2, 3)).astype(np.float16)

    e128 = np.asarray(embeds, np.float32).reshape(PLANES, H, W).astype(np.float16)
    t128 = np.asarray(tf_embeds, np.float32).reshape(PLANES, H, W).astype(np.float16)
    # padded halo: tf_h[q, p, c, j] = tf[q, (4p+c-3)%H, (j-3)%W]
    rows = (HB * np.arange(P)[:, None] + np.arange(HALO)[None, :] - 3) % H
    cols = (np.arange(WP) - 3) % W
    tf_h = np.ascontiguousarray(t128[:, rows][:, :, :, cols])  # [PLANES,P,7,WP]
    ident = np.eye(P, dtype=np.float16)
    in_maps = [
        {
            "e_sh": np.ascontiguousarray(e128[ci * PPC:(ci + 1) * PPC]),
            "tf_sh": tf_h[ci * PPC:(ci + 1) * PPC],
            "cmap": cl,
            "ident": ident,
        }
        for ci in range(NCORES)
    ]
    return in_maps, csum


def kernel(embeds, tf_embeds, raw, mask):
    from concourse.bass_utils import run_bass_kernel_spmd

    in_maps, csum = _prepare(embeds, tf_embeds, raw, mask)

    if "nc" not in _CACHE:
        _CACHE["nc"] = _build_bass()
    res = run_bass_kernel_spmd(
        _CACHE["nc"], in_maps, core_ids=list(range(NCORES)),
    )
    _CACHE["last_results"] = res

    s = np.float64(0.0)
    for om in res.results:
        s += om["partials"].astype(np.float64).sum()

    loss = (B * csum - s) / float(B * H * W)
    return np.asarray(loss, dtype=np.float32)


def benchmark(embeds, tf_embeds, raw, mask, iters=20, depth=32):
    """Time the device program with inputs resident on the 8 cores.

    Issues `depth` kernel dispatches back-to-back before blocking, so the
    device-side execution time dominates the (large, noisy) axon dispatch
    overhead; a null kernel run the same way calibrates that overhead.
    Returns (per_call_main_seconds, per_call_null_seconds) lists where each
    entry is total_time / depth for one timed round.
    """
    import time
    import jax
    import concourse.bass as bass
    import concourse.mybir as mybir
    from concourse import bass2jax
    from jax.sharding import Mesh, PartitionSpec, NamedSharding
    from jax.experimental.shard_map import shard_map

    if "nc" not in _CACHE:
        _CACHE["nc"] = _build_bass()

    def make_runner(nc, in_maps):
        pid = nc.partition_id_tensor.name if nc.partition_id_tensor else None
        in_names, out_names, out_avals, zeros = [], [], [], []
        for alloc in nc.m.functions[0].allocations:
            if type(alloc).__name__ != "MemoryLocationSet":
                continue
            name = alloc.memorylocations[0].name
            if alloc.kind == "ExternalInput":
                if name != pid:
                    in_names.append(name)
            elif alloc.kind == "ExternalOutput":
                out_names.append(name)
                shape = tuple(alloc.tensor_shape)
                dt = mybir.dt.np(alloc.dtype)
                out_avals.append(jax.core.ShapedArray(shape, dt))
                zeros.append(np.zeros(shape, dt))
        n_params = len(in_names)
        all_names = in_names + out_names + ([pid] if pid else [])

        def _body(*args):
            ops = list(args)
            if pid:
                ops.append(bass2jax.partition_id_tensor())
            return tuple(bass2jax._bass_exec_p.bind(
                *ops, out_avals=tuple(out_avals), in_names=tuple(all_names),
                out_names=tuple(out_names), lowering_input_output_aliases=(),
                sim_require_finite=True, sim_require_nnan=True, nc=nc))

        n = NCORES
        devices = jax.devices()[:n]
        mesh = Mesh(np.asarray(devices), ("core",))
        n_outs = len(out_names)
        sharded = jax.jit(
            shard_map(_body, mesh=mesh,
                      in_specs=(PartitionSpec("core"),) * (n_params + n_outs),
                      out_specs=(PartitionSpec("core"),) * n_outs,
                      check_rep=False),
            donate_argnums=tuple(range(n_params, n_params + n_outs)),
            keep_unused=True)
        sh = NamedSharding(mesh, PartitionSpec("core"))
        d_in = [jax.device_put(
                    np.concatenate([np.asarray(m[k]) for m in in_maps], axis=0), sh)
                for k in in_names]
        cz = [np.concatenate([z] * n, axis=0) for z in zeros]

        def run_round():
            dzs = [[jax.device_put(z, sh) for z in cz] for _ in range(depth)]
            for dz in dzs:
                for a in dz:
                    a.block_until_ready()
            t0 = time.perf_counter()
            outs = []
            for dz in dzs:
                outs.append(sharded(*d_in, *dz))
            for o in outs[-1]:
                o.block_until_ready()
            return (time.perf_counter() - t0) / depth
        return run_round

    in_maps, _ = _prepare(embeds, tf_embeds, raw, mask)
    run_main = make_runner(_CACHE["nc"], in_maps)

    # null kernel: copy a tiny tensor in->out, same dispatch path
    f32 = mybir.dt.float32
    nc2 = bass.Bass()
    a_in = nc2.dram_tensor("a", [P, 16], f32, kind="ExternalInput")
    b_out = nc2.dram_tensor("b", [P, 16], f32, kind="ExternalOutput")
    with (nc2.sbuf_tensor([P, 16], f32) as t,
          nc2.semaphore() as s,
          nc2.Block() as blk):
        @blk.sync
        def _(sync):
            sync.dma_start(t[:], a_in[:]).then_inc(s, 16)
            sync.wait_ge(s, 16)
            sync.dma_start(b_out[:], t[:]).then_inc(s, 16)
            sync.wait_ge(s, 32)
    null_maps = [{"a": np.zeros((P, 16), np.float32)} for _ in range(NCORES)]
    run_null = make_runner(nc2, null_maps)

    main_ts = [run_main() for _ in range(iters)]
    null_ts = [run_null() for _ in range(iters)]
    return main_ts, null_ts
